# revision 31
# baseline (speedup 1.0000x reference)
"""CrossModalAttention TRN2 kernel (v10).

Computation (per batch b):
  Q_m = x_m @ W_m ; K_m = x_m @ W_m^T   (m in {rna, cnv, clinical})
  out  = mean_i( sum_{j!=i} softmax(Q_i K_j^T / 8) @ x_j )

v7 = v2 + PV chunk-pair weight sharing + head fixes.
  POWER NOTE: a fully-densified variant (v3: chunk-interleaved scores+PV,
  ~90% PE fill duty) deterministically trips the chip power governor into
  P0 (all engines at 5/6 clock) and is NET SLOWER.  v7 keeps v2's per-slot
  shape (1 score pair + 2 PV matmuls) and only removes redundant
  LDWEIGHTS: PV matmuls run one chunk-PAIR late, t-major, so the two PV
  matmuls in a slot share the same xo[t] stationary operand.

  - Pure data parallel: batch 16 sharded 2-per-core across 8 NeuronCores.
  - Host precomputes Q/K projections; device runs the O(N^2) attention.
  - Scores ST[m, n] in PSUM via concurrent row-half matmul pairs (fp16).
  - exp split ACT (true exp, reads PSUM) / DVE (Schraudolph int16-bitcast
    fp16; sawtooth error cancels in softmax; end-to-end rel err ~2.5e-3).
  - PV contracts m on partitions; softmax denominator rides as a 65th
    column of x_j set to 3.0 (folds the 3-modality mean).
  - PV window (chunks 2k, 2k+1) streams during the score slots of chunks
    2k+2, 2k+3; slot w emits pv(cA, t=w) + pv(cB, t=w) sharing one LDW.
  - Window-end PSUM evacuation: chunk A copy on DVE, chunk B copy on ACT
    (parallel engines) so the next window's first PV isn't bank-blocked.
  - out^T chunks PE-transposed back to [n, d], normalized (reciprocal +
    broadcast mult on DVE), accumulated on GPSIMD; dribbled at w=2/w=4.
  - Head: ident DMA demoted after the first-needed score slices; PE HAM
    pre-warmed with 7 N=512 dummy matmuls on zeroed scratch during the
    DMA wait so real matmuls start at 2.4 GHz.
  - PSUM: 3 score slots (6 banks) + 2 out_ps banks = 8.
"""

import os

import numpy as np

import concourse.bass as bass
import concourse.bacc as bacc
import concourse.tile as tile
from concourse import mybir
from concourse.bass_utils import run_bass_kernel_spmd

B, N, D = 16, 2048, 64
NCORES = 8
BPC = B // NCORES  # batches per core
NT = N // 128  # 16 m-tiles of 128
CH = 512  # n-chunk (PSUM bank)
NCH = N // CH  # 4
PAIRS = [(i, j) for i in range(3) for j in range(3) if i != j]
SCALE = 1.0 / 8.0  # 1/sqrt(D)
F32 = mybir.dt.float32
F16 = mybir.dt.float16  # matmul operand dtype: 1 cyc/row
I16 = mybir.dt.int16

NG = 8  # score groups per chunk; group g covers m-tiles (2g, 2g+1)

# Schraudolph constants: int16 = round(s * SCH_A + SCH_B); bitcast fp16.
LOG2E = float(np.log2(np.e))
SCH_C = 0.0580  # minimizes rms rel err of the sawtooth
SCH_A = 1024.0 * LOG2E * SCALE
SCH_B = 1024.0 * (15.0 - SCH_C)

DVE_GROUPS = (2, 4, 6)  # exp groups handled by DVE Schraudolph; phiD = 6/16

_cache = {}
last_results = None  # BassKernelResults of the most recent run (for test.py)


def _build():
    nc = bacc.Bacc()
    qt_d = [
        nc.declare_dram_parameter(f"qt{m}", [BPC, 128, N], F16, isOutput=False)
        for m in range(3)
    ]
    kt_d = [
        nc.declare_dram_parameter(f"kt{m}", [BPC, 128, N], F16, isOutput=False)
        for m in range(3)
    ]
    xo_d = [
        nc.declare_dram_parameter(f"xo{m}", [BPC, 128, NT, D + 1], F16, isOutput=False)
        for m in range(3)
    ]
    out_d = nc.declare_dram_parameter("out", [BPC, N, D], F16, isOutput=True)

    from contextlib import ExitStack

    with tile.TileContext(nc) as tc, ExitStack() as ctx:
        singles = ctx.enter_context(tc.tile_pool(name="singles", bufs=1))
        big = ctx.enter_context(tc.tile_pool(name="big", bufs=2))
        work = ctx.enter_context(tc.tile_pool(name="work", bufs=3))
        psum = ctx.enter_context(tc.tile_pool(name="psum", bufs=3, space="PSUM"))

        # Warm up the ACT engine: absorb the exp-table load and the const
        # bias-AP DMA wait into one early instruction.
        warm = singles.tile([128, 1], F32)
        bias0 = nc.const_aps.scalar_like(0.0, warm[:, 0:1])
        nc.scalar.activation(warm, bias0, mybir.ActivationFunctionType.Exp)
        # Pre-warm the PE HAM clock gate during the head DMA wait: 7 N=512
        # dummy matmuls on zeroed scratch (~3us busy, no DMA dependency) so
        # the first real score matmuls run at 2.4 GHz instead of 1.2.
        scr = singles.tile([64, CH], F16)
        nc.vector.memset(scr, 0)
        warm_ps = psum.tile([128, CH], F32, tag="st", name="warm_ps")
        for w in range(7):
            nc.tensor.matmul(
                warm_ps, lhsT=scr[:, 0:128], rhs=scr, start=True, stop=True,
                skip_group_check=True,
            )

        for b in range(BPC):
            qt_sb, kt_sb, xo_sb = [None] * 3, [None] * 3, [None] * 3
            for m in range(3):
                qt_sb[m] = big.tile([128, N], F16, tag=f"qt{m}", name=f"qt{m}_{b}")
                kt_sb[m] = big.tile([128, N], F16, tag=f"kt{m}", name=f"kt{m}_{b}")
                xo_sb[m] = big.tile(
                    [128, NT, D + 1], F16, tag=f"xo{m}", name=f"xo{m}_{b}"
                )
            # Issue the first pair's ((0,1)) inputs first so compute can start
            # before the remaining loads land.  For b=0 (cold start, all 8
            # cores pulling HBM simultaneously) the first-needed slices go
            # first and the ident load is demoted behind them.
            if b == 0:
                nc.sync.dma_start(
                    out=kt_sb[1][:, 0:768], in_=kt_d[1][b][:, 0:768]
                )
                nc.sync.dma_start(
                    out=qt_sb[0][:, 0:CH], in_=qt_d[0][b][:, 0:CH]
                )
                nc.sync.dma_start(
                    out=kt_sb[1][:, 768:], in_=kt_d[1][b][:, 768:]
                )
                nc.sync.dma_start(
                    out=qt_sb[0][:, CH:], in_=qt_d[0][b][:, CH:]
                )
                rest = (
                    (1, xo_sb, xo_d),
                    (1, qt_sb, qt_d), (2, kt_sb, kt_d), (2, xo_sb, xo_d),
                    (2, qt_sb, qt_d), (0, kt_sb, kt_d), (0, xo_sb, xo_d),
                )
            else:
                rest = (
                    (0, qt_sb, qt_d), (1, kt_sb, kt_d), (1, xo_sb, xo_d),
                    (1, qt_sb, qt_d), (2, kt_sb, kt_d), (2, xo_sb, xo_d),
                    (2, qt_sb, qt_d), (0, kt_sb, kt_d), (0, xo_sb, xo_d),
                )
            for m, t_sb, t_d in rest:
                nc.sync.dma_start(out=t_sb[m], in_=t_d[m][b])
            acc = big.tile([128, NT, D], F16, tag="acc", name=f"acc_{b}")
            nc.vector.memset(acc, 0)

            # Flat schedule of score slots; scores are emitted THREE slots
            # ahead of the exp stream; PVs run one chunk-PAIR window late.
            sched = [
                (i, j, c, g) for (i, j) in PAIRS for c in range(NCH)
                for g in range(NG)
            ]
            st_tiles = {}
            ptt_store = {}  # (gc, g) -> exp'd fp16 view
            out_ps_store = {}  # gc -> psum accumulator

            def emit_st(idx):
                i, j, c, g = sched[idx]
                stt = psum.tile(
                    [128, 2 * CH], F32, tag="st",
                    name=f"st_{b}_{i}{j}_{c}_{g}",
                )
                st_tiles[idx] = stt
                for p in range(2):
                    t = 2 * g + p
                    h = (t % 2) * 64  # alternate PE row halves -> concurrent
                    nc.tensor.matmul(
                        stt[:, p * CH : (p + 1) * CH],
                        lhsT=kt_sb[j][h : h + 64, t * 128 : (t + 1) * 128],
                        rhs=qt_sb[i][h : h + 64, c * CH : (c + 1) * CH],
                        start=True,
                        stop=True,
                    )

            def pv_mm(gc, t, second):
                # One PV matmul for global-chunk gc, m-tile t.  Slots emit
                # (gcA, t) then (gcB, t) back to back: the second shares the
                # just-loaded xo[t] stationary operand (walrus still emits
                # the LDW, but it is a same-weights reload that overlaps).
                pi = gc // NCH
                i, j = PAIRS[pi]
                c = gc % NCH
                if t == 0:
                    out_ps_store[gc] = psum.tile(
                        [80, CH], F32, tag="out", bufs=2,
                        name=f"o_{b}_{i}{j}_{c}",
                    )
                ops = out_ps_store[gc]
                ptt = ptt_store[(gc, t // 2)]
                nc.tensor.matmul(
                    ops[0 : D + 1, :],
                    lhsT=xo_sb[j][:, t, :],
                    rhs=ptt[:, (t % 2) * CH : (t % 2 + 1) * CH],
                    start=(t == 0),
                    stop=(t == NT - 1),
                    skip_group_check=True,
                )
                if t == NT - 1:
                    # Evacuate UNNORMALIZED out^T (+3Z row) as fp16 scaled
                    # 2^-6 (cancels in the out/Z ratio; keeps 3Z in fp16
                    # range), padded to 80 rows for the 16-row XBAR tiles.
                    # DMA-transposes to [n, d] so the reciprocal runs as a
                    # 128-lane [128,4] op (a [1,512] DVE recip costs 3.3us).
                    # No PE transposes, no PSUM score-slot borrow.
                    osbu = work.tile(
                        [80, CH], F16, tag="osb", name=f"osb_{b}_{i}{j}_{c}"
                    )
                    if second:
                        nc.scalar.activation(
                            osbu, ops, mybir.ActivationFunctionType.Copy,
                            scale=2.0 ** -6,
                        )
                    else:
                        nc.vector.tensor_scalar(
                            out=osbu, in0=ops, scalar1=2.0 ** -6,
                            scalar2=0.0, op0=mybir.AluOpType.mult,
                            op1=mybir.AluOpType.add,
                        )
                    out_ps_store.pop(gc)
                    otu = work.tile(
                        [128, 4, 80], F16, tag="otn", name=f"otn_{b}_{i}{j}_{c}"
                    )
                    nc.sync.dma_start_transpose(out=otu, in_=osbu)
                    rz = work.tile([128, 4], F32, tag="rz", name=f"rz_{b}_{i}{j}_{c}")
                    nc.vector.reciprocal(rz, otu[:, :, D])
                    rz16 = work.tile(
                        [128, 4], F16, tag="rz16", name=f"rz16_{b}_{i}{j}_{c}"
                    )
                    nc.vector.tensor_copy(out=rz16, in_=rz)
                    rzb = rz16.unsqueeze(2).broadcast_to([128, 4, D])
                    res = work.tile(
                        [128, 4, D], F16, tag="res", name=f"res_{b}_{i}{j}_{c}"
                    )
                    nc.vector.tensor_tensor(
                        out=res, in0=otu[:, :, 0:D], in1=rzb,
                        op=mybir.AluOpType.mult,
                    )
                    nc.gpsimd.tensor_tensor(
                        out=acc[:, c * 4 : (c + 1) * 4, :],
                        in0=acc[:, c * 4 : (c + 1) * 4, :],
                        in1=res,
                        op=mybir.AluOpType.add,
                    )
                    if (i, j) == PAIRS[-1]:
                        nc.sync.dma_start(
                            out=out_d[b].rearrange("(t p) d -> p t d", p=128)[
                                :, c * 4 : (c + 1) * 4, :
                            ],
                            in_=acc[:, c * 4 : (c + 1) * 4, :],
                        )

            emit_st(0)
            emit_st(1)
            emit_st(2)
            for idx, (i, j, c, g) in enumerate(sched):
                gc = (idx // NG) % (len(PAIRS) * NCH)
                stt = st_tiles.pop(idx)
                # For the last two chunks of the batch, put the DVE exps on
                # the EARLY groups: the DVE queue is then empty when the
                # window-end evacuation copy (DVE) is emitted at the final
                # slot, releasing the drain's out-bank ~6-9us earlier
                # (measured 10.4us pre-drain PE gap otherwise).
                dgroups = (
                    (0, 1, 2) if gc >= len(PAIRS) * NCH - 2 else DVE_GROUPS
                )
                if g in dgroups:
                    # DVE Schraudolph exp -> int16, consumed bitcast as fp16
                    ptd = work.tile(
                        [128, 2 * CH], I16, tag="pt", bufs=34,
                        name=f"pt_{b}_{i}{j}_{c}_{g}",
                    )
                    nc.vector.tensor_scalar(
                        out=ptd, in0=stt, scalar1=SCH_A, scalar2=SCH_B,
                        op0=mybir.AluOpType.mult, op1=mybir.AluOpType.add,
                    )
                    ptt = ptd.bitcast(F16)
                else:
                    ptt = work.tile(
                        [128, 2 * CH], F16, tag="pt", bufs=34,
                        name=f"pt_{b}_{i}{j}_{c}_{g}",
                    )
                    nc.scalar.activation(
                        ptt, stt, mybir.ActivationFunctionType.Exp, scale=SCALE
                    )
                ptt_store[(gc, g)] = ptt

                # PV window: chunks (base, base+1) stream t-major during the
                # score slots of chunks (base+2, base+3).  (A tighter
                # one-slot-lag mapping trips the P0 power downclock.)
                base = (gc // 2 - 1) * 2
                if base >= 0:
                    w = (gc % 2) * NG + g  # 0..15 within the window = t
                    pv_mm(base, w, second=False)
                    pv_mm(base + 1, w, second=True)
                if idx + 3 < len(sched):
                    emit_st(idx + 3)

            # Drain: the last chunk-pair window of the batch.  Chunk A
            # fully first -- it is gated only by the now-fast DVE-side copy.
            last = len(PAIRS) * NCH - 2
            for t in range(NT):
                pv_mm(last, t, second=False)
            for t in range(NT):
                pv_mm(last + 1, t, second=True)
    nc.finalize()  # Bacc: split multi-waits, alloc regs, etc.
    return nc


def _prep(xs, Ws):
    """Host-side input prep: Q/K projections + layout shuffles."""
    qts, kts, xos = [], [], []
    for m in range(3):
        x = np.ascontiguousarray(xs[m], dtype=np.float32)  # [B, N, D]
        W = np.asarray(Ws[m], dtype=np.float32)
        Q = x @ W  # [B, N, D]
        K = x @ W.T
        QT = np.ascontiguousarray(Q.transpose(0, 2, 1))  # [B, D, N]
        KT = np.ascontiguousarray(K.transpose(0, 2, 1))
        qts.append(np.concatenate([QT, QT], axis=1).astype(np.float16))  # [B, 128, N]
        kts.append(np.concatenate([KT, KT], axis=1).astype(np.float16))
        xo = np.full((B, 128, NT, D + 1), 3.0, dtype=np.float16)
        # xo[b, p, t, :64] = x[b, t*128 + p, :]; col 64 stays 3.0 (folds the
        # mean over 3 modalities into the softmax normalization).
        xo[..., :D] = x.reshape(B, NT, 128, D).transpose(0, 2, 1, 3).astype(np.float16)
        xos.append(xo)
    return qts, kts, xos


def kernel(x_rna, x_cnv, x_clinical, W_rna, W_cnv, W_clinical):
    global last_results
    xs = [x_rna, x_cnv, x_clinical]
    Ws = [W_rna, W_cnv, W_clinical]
    qts, kts, xos = _prep(xs, Ws)
    if "nc" not in _cache:
        _cache["nc"] = _build()
    nc = _cache["nc"]

    in_maps = []
    for c in range(NCORES):
        sl = slice(c * BPC, (c + 1) * BPC)
        m = {}
        for mod in range(3):
            m[f"qt{mod}"] = np.ascontiguousarray(qts[mod][sl])
            m[f"kt{mod}"] = np.ascontiguousarray(kts[mod][sl])
            m[f"xo{mod}"] = np.ascontiguousarray(xos[mod][sl])
        in_maps.append(m)

    # The first execution on a freshly-wedged device occasionally fails with
    # NRT_EXEC_UNIT_UNRECOVERABLE; a retry on the reset device succeeds.
    attempt = 0
    while True:
        try:
            last_results = run_bass_kernel_spmd(
                nc,
                in_maps,
                list(range(NCORES)),
                trace=bool(os.environ.get("BASS_TRACE")),
            )
            break
        except Exception:
            attempt += 1
            if attempt > 2:
                raise
    out = np.concatenate([r["out"] for r in last_results.results], axis=0).astype(np.float32)
    return out
